# revision 15
# baseline (speedup 1.0000x reference)
"""Trainium2 Bass kernel for a 5-layer GPT-style transformer (BigramLanguageModel).

Sharding: data-parallel over batch (B=8 -> 1 sequence per core) through the
transformer layers (zero collectives), then AllGather of the final hidden
states and a vocab-parallel LM head (each core computes all 4096 tokens x its
4000-wide vocab shard).

Layout strategy per core:
  - residual stream h kept natural [T,D] (LN/softmax reduce over free dim)
  - LN outputs PE-transposed to [D,T] (f32r) for matmuls
  - attention scores computed TRANSPOSED ([s,t]); the causal mask is added to
    the PSUM scores (additive -1e9 on the diagonal block) BEFORE the exp so
    the attn@V matmul depends only on the exp; the softmax denominator comes
    from an appended ones-column in the V operand, reciprocated straight out
    of PSUM and applied via a ones[1,64] x recip-row broadcast matmul.
  - matmuls in float32r (full PE rate at N>=256, ~1e-4 relative rounding)
  - layer-phase SBUF pools close at the AllGather barrier, freeing ~116KB so
    the head holds the full 4000-wide Wout shard and stages full logit rows
    (single 2MB output DMA per 128 tokens, gathered activations read once)
  - LN gains==1 / biases==0 (checked at call time) skip their elementwise ops
"""

import sys

import numpy as np

sys.path.insert(0, "/opt/trn_rl_repo")

import concourse.bass as bass
import concourse.mybir as mybir
import concourse.tile as tile
from concourse import bacc
from concourse.bass_utils import run_bass_kernel_spmd

F32 = mybir.dt.float32
F32R = mybir.dt.float32r
I32 = mybir.dt.int32
AF = mybir.ActivationFunctionType
ALU = mybir.AluOpType

D, H, HS, L, V, CTX, B, T, FF = 384, 6, 64, 5, 32000, 512, 8, 512, 1536
P = 128
DT = D // P          # 3 d-tiles
TT = T // P          # 4 t-tiles
NT = FF // P         # 12 ff-tiles
N_CORES = 8
VSH = V // N_CORES   # 4000 vocab shard per core
VCH = 500            # vocab chunk per matmul (PSUM bank = 512 f32)
NCH = VSH // VCH     # 8 chunks per 128-token row
SCALE = float(D) ** -0.5
EPS = 1e-5


def _build(trivial_gb=False, trivial_bias=False, trivial_bout=False,
           sim_nocc=False):
    nc = bacc.Bacc("TRN2", target_bir_lowering=False, debug=False,
                   num_devices=1 if sim_nocc else N_CORES)

    io = {}
    io["x"] = nc.dram_tensor("x", [T], I32, kind="ExternalInput")
    io["tok_emb"] = nc.dram_tensor("tok_emb", [V, D], F32, kind="ExternalInput")
    io["pos_emb"] = nc.dram_tensor("pos_emb", [CTX, D], F32, kind="ExternalInput")
    for n, shp in [("ln1_g", [L, D]), ("ln1_b", [L, D]),
                   ("Wq", [L, D, D]), ("Wk", [L, D, D]), ("Wv", [L, D, D]),
                   ("Wproj", [L, D, D]), ("bproj", [L, D]),
                   ("ln2_g", [L, D]), ("ln2_b", [L, D]),
                   ("W1", [L, D, FF]), ("b1", [L, FF]),
                   ("W2", [L, FF, D]), ("b2", [L, D]),
                   ("lnf_g", [D]), ("lnf_b", [D]),
                   ("Wout_sh", [D, VSH]), ("bout_sh", [VSH])]:
        io[n] = nc.dram_tensor(n, shp, F32, kind="ExternalInput")
    io["logits_sh"] = nc.dram_tensor("logits_sh", [B * T, VSH], F32,
                                     kind="ExternalOutput")

    io["ident_d"] = nc.inline_tensor(np.eye(P, dtype=np.float32), name="ident_c")
    ntriu = (1.0 - np.triu(np.ones((P, P), np.float32))) * -1e9
    io["ntriu_d"] = nc.inline_tensor(ntriu.astype(np.float32), name="ntriu_c")
    io["ones64_d"] = nc.inline_tensor(np.ones((1, 64), np.float32),
                                      name="ones64_c")
    io["onesP_d"] = nc.inline_tensor(np.ones((P, 1), np.float32), name="onesP_c")

    with tile.TileContext(nc) as tc:
        _emit(nc, tc, io, trivial_gb, trivial_bias, trivial_bout, sim_nocc)
    nc.compile()
    return nc


def _emit(nc, tc, io, trivial_gb, trivial_bias, trivial_bout, sim_nocc):
    from contextlib import ExitStack
    octx = ExitStack()
    with octx:
        dram = octx.enter_context(tc.tile_pool(name="dram", bufs=1, space="DRAM"))
        pp_mm = octx.enter_context(tc.tile_pool(name="pp_mm", bufs=4,
                                                space="PSUM"))
        pp_tp = octx.enter_context(tc.tile_pool(name="pp_tp", bufs=2,
                                                space="PSUM"))
        pp_at = octx.enter_context(tc.tile_pool(name="pp_at", bufs=2,
                                                space="PSUM"))
        ag_in = dram.tile([D, T], F32, name="ag_in")
        VHALF = VSH // 2
        woutp = octx.enter_context(tc.tile_pool(name="woutp", bufs=1))
        wo_a = woutp.tile([P, DT, VHALF], F32R, name="wo_a", tag="wo_a")
        nc.sync.dma_start(
            out=wo_a[:],
            in_=io["Wout_sh"][:, 0:VHALF]
            .rearrange("(k p) n -> p k n", p=P).bitcast(F32R))

        with ExitStack() as ictx:
            const = ictx.enter_context(tc.tile_pool(name="const", bufs=1))
            hp = ictx.enter_context(tc.tile_pool(name="hp", bufs=1))
            act = ictx.enter_context(tc.tile_pool(name="act", bufs=6))
            atp = ictx.enter_context(tc.tile_pool(name="atp", bufs=2))
            qkp = ictx.enter_context(tc.tile_pool(name="qkp", bufs=1))
            vp = ictx.enter_context(tc.tile_pool(name="vp", bufs=5))
            ptp = ictx.enter_context(tc.tile_pool(name="ptp", bufs=8))
            up = ictx.enter_context(tc.tile_pool(name="up", bufs=3))
            otp = ictx.enter_context(tc.tile_pool(name="otp", bufs=1))
            gtp = ictx.enter_context(tc.tile_pool(name="gtp", bufs=2))
            wp = ictx.enter_context(tc.tile_pool(name="wp", bufs=6))
            w1p = ictx.enter_context(tc.tile_pool(name="w1p", bufs=1))
            w2p = ictx.enter_context(tc.tile_pool(name="w2p", bufs=1))
            bcp = ictx.enter_context(tc.tile_pool(name="bcp", bufs=4))
            smp = ictx.enter_context(tc.tile_pool(name="smp", bufs=4))

            ident = const.tile([P, P], F32, name="ident")
            nc.sync.dma_start(out=ident[:], in_=io["ident_d"][:])
            ntriu = const.tile([P, P], F32, name="ntriu")
            nc.sync.dma_start(out=ntriu[:], in_=io["ntriu_d"][:])
            ones64 = const.tile([1, 64], F32R, name="ones64")
            nc.sync.dma_start(out=ones64[:], in_=io["ones64_d"][:].bitcast(F32R))
            onesP = const.tile([P, 1], F32R, name="onesP")
            nc.sync.dma_start(out=onesP[:], in_=io["onesP_d"][:].bitcast(F32R))
            epsP = const.tile([P, 1], F32, name="epsP")
            nc.vector.memset(epsP[:], EPS)

            # ---- embedding gather + pos ----
            h_sb = []
            for m in range(TT):
                it = smp.tile([P, 1], I32, name=f"idx{m}", tag="idx")
                nc.sync.dma_start(out=it[:], in_=io["x"][P * m:P * (m + 1), None])
                ht = hp.tile([P, D], F32, name=f"h{m}", tag=f"h{m}")
                nc.gpsimd.indirect_dma_start(
                    out=ht[:], out_offset=None, in_=io["tok_emb"][:],
                    in_offset=bass.IndirectOffsetOnAxis(ap=it[:, :1], axis=0))
                pt = act.tile([P, D], F32, name=f"pos{m}", tag="af")
                nc.sync.dma_start(out=pt[:],
                                  in_=io["pos_emb"][P * m:P * (m + 1), :])
                nc.vector.tensor_tensor(out=ht[:], in0=ht[:], in1=pt[:],
                                        op=ALU.add)
                h_sb.append(ht)

            def layer_norm(src_tiles, g_dram, b_dram, tag):
                if not trivial_gb:
                    g_bc = bcp.tile([P, D], F32, name=f"g_{tag}", tag="gb")
                    nc.sync.dma_start(out=g_bc[:],
                                      in_=g_dram[None, :].to_broadcast([P, D]))
                    b_bc = bcp.tile([P, D], F32, name=f"b_{tag}", tag="gb")
                    nc.sync.dma_start(out=b_bc[:],
                                      in_=b_dram[None, :].to_broadcast([P, D]))
                outs = []
                for m in range(TT):
                    st = smp.tile([P, 6], F32, name=f"st_{tag}{m}", tag="st")
                    nc.vector.bn_stats(out=st[:], in_=src_tiles[m][:])
                    mv = smp.tile([P, 2], F32, name=f"mv_{tag}{m}", tag="mv")
                    nc.vector.bn_aggr(out=mv[:], in_=st[:])
                    nc.scalar.activation(out=mv[:, 1:2], in_=mv[:, 1:2],
                                         func=AF.Sqrt, bias=epsP[:])
                    nc.vector.reciprocal(out=mv[:, 1:2], in_=mv[:, 1:2])
                    at = act.tile([P, D], F32, name=f"a_{tag}{m}", tag="af")
                    nc.vector.tensor_scalar(out=at[:], in0=src_tiles[m][:],
                                            scalar1=mv[:, 0:1],
                                            scalar2=mv[:, 1:2],
                                            op0=ALU.subtract, op1=ALU.mult)
                    if not trivial_gb:
                        nc.vector.tensor_tensor(out=at[:], in0=at[:], in1=g_bc[:],
                                                op=ALU.mult)
                        nc.vector.tensor_tensor(out=at[:], in0=at[:], in1=b_bc[:],
                                                op=ALU.add)
                    outs.append(at)
                return outs

            def transpose_to(a_tiles, tag):
                """[TT][P,D] natural f32 -> [DT][P,T] f32r transposed"""
                outs = []
                for k in range(DT):
                    ps = pp_tp.tile([P, T], F32, name=f"tp_{tag}{k}", tag="tp")
                    for m in range(TT):
                        nc.tensor.transpose(ps[:, P * m:P * (m + 1)],
                                            a_tiles[m][:, P * k:P * (k + 1)],
                                            ident[:])
                    dst = atp.tile([P, T], F32R, name=f"{tag}T{k}", tag=f"aT{k}")
                    nc.scalar.copy(dst[:], ps[:])
                    outs.append(dst)
                return outs

            def load_w_dxd(w_dram, l, tag):
                """W[l] [D,D] -> one [P, DT, D] f32r tile (single DMA)"""
                wt = wp.tile([P, DT, D], F32R, name=f"{tag}{l}", tag="wdd")
                nc.sync.dma_start(
                    out=wt[:],
                    in_=w_dram[l].rearrange("(k p) n -> p k n", p=P).bitcast(F32R))
                return wt

            # ================= layers =================
            for l in range(L):
                a_t = layer_norm(h_sb, io["ln1_g"][l, :], io["ln1_b"][l, :],
                                 f"l{l}n1")
                aT = transpose_to(a_t, f"l{l}a")

                wq_t = load_w_dxd(io["Wq"], l, "wq")
                wk_t = load_w_dxd(io["Wk"], l, "wk")
                wv_t = load_w_dxd(io["Wv"], l, "wv")
                wpj_t = load_w_dxd(io["Wproj"], l, "wp")

                qT, kT = [], []
                for name, wt, dstl in (("q", wq_t, qT), ("k", wk_t, kT)):
                    for dq in range(DT):
                        ps = pp_mm.tile([P, T], F32, name=f"ps_{name}{l}{dq}",
                                        tag="mm")
                        for d in range(DT):
                            nc.tensor.matmul(ps[:], wt[:, d, P * dq:P * (dq + 1)],
                                             aT[d][:], start=(d == 0),
                                             stop=(d == DT - 1))
                        dst = qkp.tile([P, T], F32R, name=f"{name}T{l}{dq}",
                                       tag=f"{name}T{dq}")
                        nc.vector.tensor_copy(out=dst[:], in_=ps[:])
                        dstl.append(dst)
                v_sb = []
                for m in range(TT):
                    ps = pp_mm.tile([P, D], F32, name=f"ps_v{l}{m}", tag="mm")
                    for d in range(DT):
                        nc.tensor.matmul(ps[:], aT[d][:, P * m:P * (m + 1)],
                                         wv_t[:, d, :], start=(d == 0),
                                         stop=(d == DT - 1))
                    vt = vp.tile([P, H, HS + 1], F32R, name=f"v{l}{m}", tag="v")
                    nc.scalar.copy(vt[:, :, 0:HS],
                                   ps[:].rearrange("p (h d) -> p h d", h=H))
                    nc.vector.tensor_copy(out=vt[:, :, HS],
                                          in_=onesP[:, 0:1].to_broadcast([P, H]))
                    v_sb.append(vt)

                # attention per head -> oT [DT][P,T] f32r
                oT = [otp.tile([P, T], F32R, name=f"oT{l}{k}", tag=f"oT{k}")
                      for k in range(DT)]
                for h in range(H):
                    r, off = (h * HS) // P, (h * HS) % P
                    pT = []
                    for j in range(TT):
                        n_j = T - P * j
                        ps = pp_mm.tile([P, T], F32, name=f"ps_s{l}{h}{j}",
                                        tag="mm")
                        nc.tensor.matmul(
                            ps[:, 0:n_j],
                            kT[r][off:off + HS, P * j:P * (j + 1)],
                            qT[r][off:off + HS, P * j:T],
                            start=True, stop=True)
                        # causal mask: additive -1e9 on the diagonal block,
                        # applied in PSUM before the exp
                        nc.vector.tensor_tensor(out=ps[:, 0:P], in0=ps[:, 0:P],
                                                in1=ntriu[:], op=ALU.add)
                        pt = ptp.tile([P, T], F32R, name=f"pT{l}{h}{j}",
                                      tag="pT")
                        nc.scalar.activation(out=pt[:, 0:n_j], in_=ps[:, 0:n_j],
                                             func=AF.Exp, scale=SCALE)
                        pT.append(pt)
                    ups = pp_at.tile([HS + 1, T], F32, name=f"ups{l}{h}",
                                     tag="at")
                    for j in range(TT):
                        n_j = T - P * j
                        nc.tensor.matmul(ups[:, P * j:T], v_sb[j][:, h, :],
                                         pT[j][:, 0:n_j],
                                         start=(j == 0), stop=(j == TT - 1))
                    u = up.tile([HS, T], F32, name=f"u{l}{h}", tag="u")
                    nc.vector.tensor_copy(out=u[:], in_=ups[0:HS, :])
                    rec = smp.tile([1, T], F32R, name=f"rec{l}{h}", tag="rec")
                    with nc.allow_low_precision(reason="softmax denom in f32r"):
                        nc.vector.reciprocal(out=rec[:], in_=ups[HS:HS + 1, :])
                    bc = pp_at.tile([64, T], F32, name=f"bc{l}{h}", tag="at")
                    nc.tensor.matmul(bc[:], ones64[:], rec[:], start=True,
                                     stop=True)
                    nc.vector.tensor_tensor(out=oT[r][off:off + HS, :],
                                            in0=u[:], in1=bc[:], op=ALU.mult)

                # proj + residual (natural out)
                if not trivial_bias:
                    bp_bc = bcp.tile([P, D], F32, name=f"bp{l}", tag="gb")
                    nc.sync.dma_start(
                        out=bp_bc[:],
                        in_=io["bproj"][l, None, :].to_broadcast([P, D]))
                for m in range(TT):
                    ps = pp_mm.tile([P, D], F32, name=f"ps_pj{l}{m}", tag="mm")
                    for d in range(DT):
                        nc.tensor.matmul(ps[:], oT[d][:, P * m:P * (m + 1)],
                                         wpj_t[:, d, :], start=(d == 0),
                                         stop=(d == DT - 1))
                    if not trivial_bias:
                        nc.vector.tensor_tensor(out=ps[:], in0=ps[:],
                                                in1=bp_bc[:], op=ALU.add)
                    nc.vector.tensor_tensor(out=h_sb[m][:], in0=h_sb[m][:],
                                            in1=ps[:], op=ALU.add)

                # ---- FF ----
                f_t = layer_norm(h_sb, io["ln2_g"][l, :], io["ln2_b"][l, :],
                                 f"l{l}n2")
                fT = transpose_to(f_t, f"l{l}f")

                w1_t = w1p.tile([P, DT, FF], F32R, name=f"w1_{l}", tag="w1")
                nc.sync.dma_start(
                    out=w1_t[:],
                    in_=io["W1"][l].rearrange("(k p) n -> p k n",
                                              p=P).bitcast(F32R))
                b1c = smp.tile([P, NT], F32, name=f"b1c{l}", tag="b1c")
                nc.sync.dma_start(
                    out=b1c[:], in_=io["b1"][l, :].rearrange("(n p) -> p n", p=P))
                w2_t = w2p.tile([P, NT, D], F32R, name=f"w2_{l}", tag="w2")
                nc.sync.dma_start(
                    out=w2_t[:],
                    in_=io["W2"][l].rearrange("(k p) n -> p k n",
                                              p=P).bitcast(F32R))

                ps_h = [pp_mm.tile([P, D], F32, name=f"ps_ff{l}{m}", tag="mm")
                        for m in range(TT)]
                for nt in range(NT):
                    psg = pp_tp.tile([P, T], F32, name=f"ps_g{l}{nt}", tag="tp")
                    for d in range(DT):
                        nc.tensor.matmul(psg[:], w1_t[:, d, P * nt:P * (nt + 1)],
                                         fT[d][:], start=(d == 0),
                                         stop=(d == DT - 1))
                    gt = gtp.tile([P, T], F32R, name=f"g{l}{nt}", tag="g")
                    nc.scalar.activation(out=gt[:], in_=psg[:], func=AF.Relu,
                                         bias=b1c[:, nt:nt + 1])
                    for m in range(TT):
                        nc.tensor.matmul(ps_h[m][:], gt[:, P * m:P * (m + 1)],
                                         w2_t[:, nt, :], start=(nt == 0),
                                         stop=(nt == NT - 1))
                if not trivial_bias:
                    b2_bc = bcp.tile([P, D], F32, name=f"b2{l}", tag="gb")
                    nc.sync.dma_start(
                        out=b2_bc[:],
                        in_=io["b2"][l, None, :].to_broadcast([P, D]))
                for m in range(TT):
                    if not trivial_bias:
                        nc.vector.tensor_tensor(out=ps_h[m][:], in0=ps_h[m][:],
                                                in1=b2_bc[:], op=ALU.add)
                    nc.vector.tensor_tensor(out=h_sb[m][:], in0=h_sb[m][:],
                                            in1=ps_h[m][:], op=ALU.add)

            # ---- final LN -> transposed -> DRAM bounce ----
            hf_t = layer_norm(h_sb, io["lnf_g"][:], io["lnf_b"][:], "lnf")
            hfT = transpose_to(hf_t, "hf")
            for k in range(DT):
                nc.sync.dma_start(out=ag_in[P * k:P * (k + 1), :],
                                  in_=hfT[k][:].bitcast(F32))
        # layer-phase SBUF pools closed here (AllGather is the barrier anyway)

        if sim_nocc:
            ag_out = dram.tile([N_CORES * D, T], F32, name="ag_out")
            for rr in range(N_CORES):
                nc.sync.dma_start(out=ag_out[rr * D:(rr + 1) * D, :],
                                  in_=ag_in[:])
        else:
            ag_out = dram.tile([N_CORES * D, T], F32, name="ag_out",
                               addr_space="Shared")
            nc.gpsimd.collective_compute(
                "AllGather", ALU.bypass,
                replica_groups=[list(range(N_CORES))],
                ins=[ag_in[:].opt()], outs=[ag_out[:].opt()])

        # ================= vocab-parallel head (full width) =================
        hfp = octx.enter_context(tc.tile_pool(name="hfp", bufs=4))
        lop = octx.enter_context(tc.tile_pool(name="lop", bufs=3))
        boutp = octx.enter_context(tc.tile_pool(name="boutp", bufs=1))

        wo_b = woutp.tile([P, DT, VHALF], F32R, name="wo_b", tag="wo_b")
        nc.sync.dma_start(
            out=wo_b[:],
            in_=io["Wout_sh"][:, VHALF:]
            .rearrange("(k p) n -> p k n", p=P).bitcast(F32R))

        def wo_slice(d, nb):
            c0 = VCH * nb
            if c0 < VHALF:
                return wo_a[:, d, c0:c0 + VCH]
            return wo_b[:, d, c0 - VHALF:c0 - VHALF + VCH]
        if not trivial_bout:
            bo_bc = boutp.tile([P, VSH], F32, name="bo", tag="bo")
            nc.sync.dma_start(
                out=bo_bc[:], in_=io["bout_sh"][None, :].to_broadcast([P, VSH]))
        ps_pools = [pp_mm] * 4 + [pp_tp] * 2 + [pp_at] * 2
        ps_tags = ["mm"] * 4 + ["tp"] * 2 + ["at"] * 2
        for b in range(N_CORES):
            hb = hfp.tile([P, DT, T], F32R, name=f"hf{b}", tag="hf")
            nc.sync.dma_start(
                out=hb[:],
                in_=ag_out[b * D:(b + 1) * D, :]
                .rearrange("(k p) n -> p k n", p=P).bitcast(F32R))
            for m in range(TT):
                row0 = b * T + P * m
                lo = lop.tile([P, VSH], F32, name=f"lo{b}{m}", tag="lo")
                for nb in range(NCH):
                    ps = ps_pools[nb].tile([P, VCH], F32,
                                           name=f"ps_o{b}{m}{nb}",
                                           tag=ps_tags[nb])
                    for d in range(DT):
                        nc.tensor.matmul(ps[:], hb[:, d, P * m:P * (m + 1)],
                                         wo_slice(d, nb),
                                         start=(d == 0), stop=(d == DT - 1))
                    sl = lo[:, VCH * nb:VCH * (nb + 1)]
                    if trivial_bout:
                        # alternate eviction engine: ACT and DVE each take half
                        if nb % 2 == 0:
                            nc.scalar.copy(sl, ps[:])
                        else:
                            nc.vector.tensor_copy(out=sl, in_=ps[:])
                    else:
                        nc.vector.tensor_tensor(
                            out=sl, in0=ps[:],
                            in1=bo_bc[:, VCH * nb:VCH * (nb + 1)], op=ALU.add)
                nc.sync.dma_start(out=io["logits_sh"][row0:row0 + P, :],
                                  in_=lo[:])


_NC_CACHE = {}


def _get_nc(trivial_gb=False, trivial_bias=False, trivial_bout=False):
    key = (trivial_gb, trivial_bias, trivial_bout)
    if key not in _NC_CACHE:
        _NC_CACHE[key] = _build(*key)
    return _NC_CACHE[key]


def _build_sim():
    return _build(trivial_gb=True, trivial_bias=True, trivial_bout=True,
                  sim_nocc=True)


def kernel(**inputs):
    inp = {k: np.ascontiguousarray(np.asarray(v)) for k, v in inputs.items()}
    trivial_gb = all(
        np.all(inp[g] == 1.0) and np.all(inp[b] == 0.0)
        for g, b in [("ln1_g", "ln1_b"), ("ln2_g", "ln2_b"), ("lnf_g", "lnf_b")])
    trivial_bias = all(np.all(inp[b] == 0.0) for b in ("bproj", "b2"))
    trivial_bout = bool(np.all(inp["bout"] == 0.0))
    nc = _get_nc(trivial_gb, trivial_bias, trivial_bout)
    in_maps = []
    for c in range(N_CORES):
        m = {
            "x": inp["x"][c].astype(np.int32),
            "tok_emb": inp["tok_emb"], "pos_emb": inp["pos_emb"],
            "ln1_g": inp["ln1_g"], "ln1_b": inp["ln1_b"],
            "Wq": inp["Wq"], "Wk": inp["Wk"], "Wv": inp["Wv"],
            "Wproj": inp["Wproj"], "bproj": inp["bproj"],
            "ln2_g": inp["ln2_g"], "ln2_b": inp["ln2_b"],
            "W1": inp["W1"], "b1": inp["b1"], "W2": inp["W2"], "b2": inp["b2"],
            "lnf_g": inp["lnf_g"], "lnf_b": inp["lnf_b"],
            "Wout_sh": np.ascontiguousarray(inp["Wout"][:, c * VSH:(c + 1) * VSH]),
            "bout_sh": np.ascontiguousarray(inp["bout"][c * VSH:(c + 1) * VSH]),
        }
        in_maps.append(m)
    res = run_bass_kernel_spmd(nc, in_maps, core_ids=list(range(N_CORES)))
    parts = [res.results[c]["logits_sh"].reshape(B, T, VSH) for c in range(N_CORES)]
    return np.concatenate(parts, axis=2)
